# revision 69
# baseline (speedup 1.0000x reference)
"""Trainium2 Bass kernel for nn_BoundLoss (pull/push embedding loss, segment_reduce).

Strategy: pure data parallel, 1 image per NeuronCore (B=8, 8 cores).
All reductions on device. Output per core: (loss_pull, loss_push) scalars.

Key ideas:
  - Segment sums (by gt_kernels / gt_texts, M=16 ids) via block-diagonal
    one-hot matmuls on the tensor engine: J pixel-column groups share one
    stationary-weight load; off-diagonal products land in PSUM cells we
    never read.
  - The per-pixel gather of centroid stats G[tt[n]] is computed in a
    (slot j, id m) partition layout: a block-diagonal G-matrix matmul
    produces, for every pixel and every id m,
    z_m = s2 - 2*dot(sim, G[m]) + g2[m] at PSUM partition j*16+m (pixel
    row = 16j+u, u indexing the moving columns).  Selection of the right
    m is ONE fused scalar_tensor_tensor per block:
    (tt_replicated == partition_id%16) * z_m, reading PSUM directly;
    the sum over the 16 m-partitions (one survivor per pixel) goes back
    through the PE with per-u one-hot stationaries, accumulated in
    bank-isolated PSUM regions.  tt_replicated is staged by a pair of
    DMAs (partition fold + stride-0 16x broadcast).  This avoids the
    16-pass select and the 13 MB dq partition-shuffle of the naive
    layout.
  - l = log1p(relu(sqrt(z)-0.5)^2) chain on the scalar engine in
    half-plane function-major passes (few ACT table reloads), emitted
    inside the phase-2 column loop so the tt segment-sum matmuls start
    while later columns are still being processed.
  - The push loss (depends only on G/valid) is computed before phase 2
    and overlaps it; only the pull-loss epilogue trails the seg-sums.
  - DMA issue is spread across SP/Act HWDGE and Pool SWDGE queues;
    input converts are spread across DVE/Pool/ACT.
"""

import os
import numpy as np
from contextlib import ExitStack

EPS = 1e-12

FULL_CFG = dict(H=640, W=640)

_CACHE = {}


def _cfg(H, W):
    P = 128
    N = H * W
    F = N // P
    assert F * P == N
    if F % 400 == 0 and F >= 1600:
        FQ = 800
        FC = 400
    elif F % 640 == 0 and F >= 1600:
        FQ = 640
        FC = 320
    else:
        FC = F // 4 if F % 4 == 0 and F // 4 <= 512 else F
        if FC > 512:
            raise ValueError("bad FC")
        FQ = F // max(1, F // (2 * FC))
    NQ = F // FQ
    assert FQ % FC == 0 and F % FQ == 0
    OHC = min(F, 400)
    assert F % OHC == 0
    # phase-2a chunking: CB cols per mask/rhs chunk, CH per PSUM D-hat block
    # (CH*4B must divide the 2KB PSUM bank so matmul outputs stay in-bank)
    CB = 640 if F % 640 == 0 else FC
    CH = 128 if CB % 128 == 0 else CB
    # phase-2b chunking
    FQT = 400 if F % 400 == 0 else FQ
    # PE group sizes (pixel columns per stationary-weight load)
    JK = 25 if OHC % 25 == 0 else 8   # kt family: 5 ch -> 125 weight cols
    JT = 32 if FQT % 32 == 0 else (25 if FQT % 25 == 0 else 8)
    assert OHC % JK == 0 and FQT % JT == 0 and F % FQT == 0
    return dict(H=H, W=W, P=P, N=N, F=F, FC=FC, FQ=FQ, NQ=NQ, OHC=OHC,
                CB=CB, CH=CH, FQT=FQT, JK=JK, JT=JT, M=16)


def build(cfg, for_sim=False):
    import concourse.bass as bass
    import concourse.bacc as bacc
    import concourse.tile as tile
    from concourse import mybir

    dt = mybir.dt
    Alu = mybir.AluOpType
    Act = mybir.ActivationFunctionType
    AX = mybir.AxisListType

    P, F, M = cfg["P"], cfg["F"], cfg["M"]
    FC, FQ, NQ, OHC = cfg["FC"], cfg["FQ"], cfg["NQ"], cfg["OHC"]
    JK, JT = cfg["JK"], cfg["JT"]

    nc = bacc.Bacc("TRN2", target_bir_lowering=False, debug=for_sim)

    sim_d = nc.dram_tensor("sim", [4, P, F], dt.float32, kind="ExternalInput")
    kt_d = nc.dram_tensor("kt", [P, F], dt.int32, kind="ExternalInput")
    tt_d = nc.dram_tensor("tt", [P, F], dt.int32, kind="ExternalInput")
    ident_d = nc.dram_tensor("ident16", [16, 16], dt.float32, kind="ExternalInput")
    iu_d = nc.dram_tensor("iu16", [16, 16], dt.float32, kind="ExternalInput")
    mge1_d = nc.dram_tensor("mge1", [16, 1], dt.float32, kind="ExternalInput")
    e6_d = nc.dram_tensor("e6", [6, 48], dt.bfloat16, kind="ExternalInput")
    dmask_d = nc.dram_tensor("dmask48", [48, 128], dt.bfloat16,
                             kind="ExternalInput")
    dselk_d = nc.dram_tensor("dselk", [JK * 5, 16 * JK], dt.bfloat16,
                             kind="ExternalInput")
    c5_d = nc.dram_tensor("c5", [JK * 5, 5], dt.float32, kind="ExternalInput")
    dselt_d = nc.dram_tensor("dselt", [JT * 2, 16 * JT], dt.bfloat16,
                             kind="ExternalInput")
    c2_d = nc.dram_tensor("c2", [JT * 2, 2], dt.float32, kind="ExternalInput")
    mvec_d = nc.dram_tensor("mvec", [128, 1], dt.float32, kind="ExternalInput")
    w8_d = nc.dram_tensor("w8all", [128, 16, 128], dt.bfloat16,
                          kind="ExternalInput")
    out_d = nc.dram_tensor("out", [1, 2], dt.float32, kind="ExternalOutput")

    with ExitStack() as ctx:
        tc = ctx.enter_context(tile.TileContext(nc, trace_sim=for_sim))

        big = ctx.enter_context(tc.tile_pool(name="big", bufs=1))
        t16 = ctx.enter_context(tc.tile_pool(name="t16", bufs=1))
        pst = ctx.enter_context(tc.tile_pool(name="pst", bufs=1, space="PSUM"))

        _tiny_n = [0]

        def tiny_ps(shape):
            _tiny_n[0] += 1
            return pst.tile(shape, dt.float32, tag="tiny",
                            name=f"tinyps{_tiny_n[0]}")

        # ---- constants (issue spread across engine DGE queues) ----
        ident16 = big.tile([16, 16], dt.float32)
        nc.scalar.dma_start(out=ident16[:], in_=ident_d.ap())
        iu16 = big.tile([16, 16], dt.float32)
        nc.scalar.dma_start(out=iu16[:], in_=iu_d.ap())
        mge1 = t16.tile([16, 1], dt.float32)
        nc.scalar.dma_start(out=mge1[:], in_=mge1_d.ap())
        e6 = big.tile([6, 48], dt.bfloat16)
        nc.sync.dma_start(out=e6[:], in_=e6_d.ap())
        dmask48 = big.tile([48, 128], dt.bfloat16)
        nc.sync.dma_start(out=dmask48[:], in_=dmask_d.ap())
        dselk = big.tile([JK * 5, 16 * JK], dt.bfloat16)
        nc.sync.dma_start(out=dselk[:], in_=dselk_d.ap())
        c5 = big.tile([JK * 5, 5], dt.float32)
        nc.scalar.dma_start(out=c5[:], in_=c5_d.ap())
        dselt = big.tile([JT * 2, 16 * JT], dt.bfloat16)
        nc.sync.dma_start(out=dselt[:], in_=dselt_d.ap())
        c2 = big.tile([JT * 2, 2], dt.float32)
        nc.scalar.dma_start(out=c2[:], in_=c2_d.ap())
        mvec = t16.tile([128, 1], dt.float32)
        nc.scalar.dma_start(out=mvec[:], in_=mvec_d.ap())
        w8all = big.tile([128, 16, 128], dt.bfloat16)
        nc.sync.dma_start(out=w8all[:], in_=w8_d.ap())
        ones16 = t16.tile([16, 1], dt.float32)
        nc.vector.memset(ones16[:], 1.0)
        b_eps = t16.tile([128, 1], dt.float32)
        nc.vector.memset(b_eps[:], EPS)
        b_mhalf = t16.tile([128, 1], dt.float32)
        nc.vector.memset(b_mhalf[:], -0.5)
        b_three = t16.tile([128, 1], dt.float32)
        nc.vector.memset(b_three[:], 3.0)

        # ---- persistent planes ----
        # simbf5: plane-major (sim0..3, ones). Serves BOTH the phase-1
        # stationary reads (via a strided f-major/ch-minor AP) and the
        # rhs_h plane DMAs.
        simbf5 = big.tile([P, 5, F], dt.bfloat16)
        vpl2I = big.tile([P, F, 2], dt.bfloat16)  # (l, ones) interleaved
        s2bf = big.tile([P, F], dt.bfloat16)
        ttbf = big.tile([P, F], dt.bfloat16)
        tt8 = big.tile([P, F], dt.float8e4)  # exact for ids 0..15
        gblock = big.tile([48, 128], dt.bfloat16)
        ktp_ctx = ExitStack()
        ktp = ktp_ctx.enter_context(tc.tile_pool(name="ktp", bufs=1))
        ktbf = ktp.tile([P, F], dt.bfloat16)
        vplI = ktp.tile([P, F, 5], dt.bfloat16)  # (sim0..3, ones) interleaved

        nc.gpsimd.memset(simbf5[:, 4, :], 1.0)
        nc.gpsimd.memset(vplI[:, :, 4], 1.0)
        nc.gpsimd.memset(vpl2I[:, :, 1], 1.0)

        # ---- load + convert inputs; s2 ----
        with tc.tile_pool(name="ld", bufs=2) as ld, \
             tc.tile_pool(name="lsq", bufs=1) as lsq, \
             tc.tile_pool(name="lds", bufs=2) as lds:
            kti = ld.tile([P, F], dt.int32, tag="idx")
            nc.scalar.dma_start(out=kti[:], in_=kt_d.ap())
            nc.vector.tensor_copy(out=ktbf[:], in_=kti[:])
            sqs = []
            for c in range(4):
                sf = lds.tile([P, F], dt.float32, tag="simf")
                nc.sync.dma_start(out=sf[:], in_=sim_d.ap()[c])
                eng2 = nc.gpsimd if c % 2 == 0 else nc.vector
                eng2.tensor_copy(out=simbf5[:, c, :], in_=sf[:])
                for hf in range(2):
                    dst = vplI[:, hf * (F // 2):(hf + 1) * (F // 2), c]
                    srch = sf[:, hf * (F // 2):(hf + 1) * (F // 2)]
                    if (2 * c + hf) % 3 == 2:
                        nc.gpsimd.tensor_copy(out=dst, in_=srch)
                    else:
                        nc.scalar.activation(out=dst, in_=srch, func=Act.Copy)
                sq = lsq.tile([P, 2, F // 2], dt.bfloat16, tag=f"sq{c % 2}")
                for hf in range(2):
                    h0, h1 = hf * (F // 2), (hf + 1) * (F // 2)
                    nc.scalar.activation(out=sq[:, hf, :], in_=sf[:, h0:h1],
                                         func=Act.Square)
                sqs.append(sq)
                if c == 1:
                    nc.vector.tensor_add(s2bf[:], sqs[0][:], sqs[1][:])
                elif c >= 2:
                    nc.vector.tensor_add(s2bf[:], s2bf[:], sqs[c][:])
            tti = ld.tile([P, F], dt.int32, tag="idx")
            nc.sync.dma_start(out=tti[:], in_=tt_d.ap())
            nc.gpsimd.tensor_copy(out=ttbf[:], in_=tti[:])
            nc.gpsimd.tensor_copy(out=tt8[:], in_=tti[:])

        # ---- phase 1: kernel-id segment sums ----
        # stationary = simbf5 strided AP [128, (f: JK, ch: 5)] (f-major,
        # ch-minor cols, same col order as the old interleaved layout);
        # moving = onehot [128, (m, J')]; psum [(J,c), (m,J')]; diagonal
        # J==J' is wanted.
        NGK = F // JK

        def ph1_lhsT(g):
            return vplI[:, g:g + JK, :]

        skC_ps = tiny_ps([5, 16 * JK])
        with tc.tile_pool(name="ps1", bufs=1, space="PSUM") as psum1:
            p1 = psum1.tile([JK * 5, 16 * JK], dt.float32)
            with tc.tile_pool(name="ohk", bufs=2) as ohkp:
                for blk in range(F // OHC):
                    ohk = ohkp.tile([P, M, OHC], dt.bfloat16, tag="ohk")
                    for m in range(M):
                        nc.vector.tensor_scalar(
                            out=ohk[:, m, :],
                            in0=ktbf[:, blk * OHC:(blk + 1) * OHC],
                            scalar1=float(m), scalar2=None, op0=Alu.is_equal)
                    for t in range(OHC // JK):
                        g = blk * OHC + t * JK
                        rhs = ohk[:, :, t * JK:(t + 1) * JK]
                        nc.tensor.matmul(p1[:], ph1_lhsT(g), rhs,
                                         start=(g == 0), stop=(g == F - JK))
            # diagonal extraction: mask off-diag, sum rows per channel via
            # matmul, then strided-reduce over J'.
            p1m = big.tile([JK * 5, 16 * JK], dt.float32)
            nc.vector.tensor_mul(p1m[:], p1[:], dselk[:])
            nc.tensor.matmul(skC_ps[:], c5[:], p1m[:], start=True, stop=True)
        skC = big.tile([5, 16], dt.float32)
        nc.vector.tensor_reduce(
            out=skC[:],
            in_=skC_ps.rearrange("p (m j) -> p m j", j=JK),
            axis=AX.X, op=Alu.add)
        skT_ps = tiny_ps([16, 5])
        nc.tensor.transpose(skT_ps[:], skC[:], ident16[0:5, 0:5])
        sk = big.tile([16, 5], dt.float32)
        nc.vector.tensor_copy(out=sk[:], in_=skT_ps[:])
        ktp_ctx.close()

        # ---- stats: G, g2, valid, Ghat-block ----
        cntk_c = t16.tile([16, 1], dt.float32)
        nc.vector.tensor_scalar(out=cntk_c[:], in0=sk[:, 4:5], scalar1=1.0,
                                scalar2=None, op0=Alu.max)
        rck = t16.tile([16, 1], dt.float32)
        nc.vector.reciprocal(rck[:], cntk_c[:])
        G = t16.tile([16, 4], dt.float32)
        nc.vector.tensor_scalar(out=G[:], in0=sk[:, 0:4], scalar1=rck[:, 0:1],
                                scalar2=None, op0=Alu.mult)
        gsq = t16.tile([16, 4], dt.float32)
        nc.scalar.activation(out=gsq[:], in_=G[:], func=Act.Square)
        g2 = t16.tile([16, 1], dt.float32)
        nc.vector.tensor_reduce(out=g2[:], in_=gsq[:], axis=AX.X, op=Alu.add)

        gext = t16.tile([16, 6], dt.float32)
        nc.vector.tensor_scalar(out=gext[:, 0:4], in0=G[:], scalar1=-2.0,
                                scalar2=None, op0=Alu.mult)
        nc.vector.tensor_copy(out=gext[:, 4:5], in_=g2[:])
        nc.vector.memset(gext[:, 5:6], 1.0)

        gsT = tiny_ps([6, 16])
        nc.tensor.transpose(gsT[:], gext[:], ident16[:])
        gsb = big.tile([6, 16], dt.bfloat16)
        nc.vector.tensor_copy(out=gsb[:], in_=gsT[:])
        # gblock[ch*8+j', c=j*16+m] = delta(j,j') * Ghat[ch, m]:
        # broadcast Ghat tiled 8x (j-major cols), expand partition groups
        # via E6 matmul, then mask the slot-block diagonal.
        gbig_row = big.tile([6, 128], dt.bfloat16)
        ga = gsb[:]
        gbc_ap = bass.AP(tensor=ga.tensor, offset=ga.offset,
                         ap=[list(ga.ap[0]), [0, 8], list(ga.ap[1])])
        nc.vector.tensor_copy(out=gbig_row[:], in_=gbc_ap)
        gbig_ps = tiny_ps([48, 128])
        nc.tensor.matmul(gbig_ps[:], e6[:], gbig_row[:], start=True, stop=True)
        nc.vector.tensor_mul(gblock[:], gbig_ps[:], dmask48[:])

        vg = t16.tile([16, 1], dt.float32)
        nc.vector.tensor_scalar(out=vg[:], in0=sk[:, 4:5], scalar1=0.0,
                                scalar2=None, op0=Alu.is_gt)
        valid = t16.tile([16, 1], dt.float32)
        nc.vector.tensor_mul(valid[:], vg[:], mge1[:])

        # ---- nv + push loss (depend only on G/valid; overlap phase 2) ----
        nv_ps = tiny_ps([1, 1])
        nc.tensor.matmul(nv_ps[:], valid[:], ones16[:], start=True, stop=True)
        nv_s = t16.tile([1, 1], dt.float32)
        nc.vector.tensor_copy(out=nv_s[:], in_=nv_ps[:])
        nv_c = t16.tile([1, 1], dt.float32)
        nc.vector.tensor_scalar(out=nv_c[:], in0=nv_s[:], scalar1=1.0,
                                scalar2=None, op0=Alu.max)
        rnv = t16.tile([1, 1], dt.float32)
        nc.vector.reciprocal(rnv[:], nv_c[:])

        # ---- push loss ----
        ones1x16 = big.tile([1, 16], dt.float32)
        nc.vector.memset(ones1x16[:], 1.0)
        gT_ps = tiny_ps([4, 16])
        nc.tensor.transpose(gT_ps[:], G[:], ident16[:])
        gt_sb = big.tile([4, 16], dt.float32)
        nc.vector.tensor_copy(out=gt_sb[:], in_=gT_ps[:])
        g2r_ps = tiny_ps([1, 16])
        nc.tensor.transpose(g2r_ps[:], g2[:], ident16[:])
        g2row = big.tile([1, 16], dt.float32)
        nc.vector.tensor_copy(out=g2row[:], in_=g2r_ps[:])
        mgt2 = big.tile([4, 16], dt.float32)
        nc.vector.tensor_scalar(out=mgt2[:], in0=gt_sb[:], scalar1=-2.0,
                                scalar2=None, op0=Alu.mult)
        dk2_ps = tiny_ps([16, 16])
        nc.tensor.matmul(dk2_ps[:], mgt2[:], gt_sb[:], start=True, stop=False)
        nc.tensor.matmul(dk2_ps[:], ones1x16[:], g2row[:], start=False,
                         stop=False)
        nc.tensor.matmul(dk2_ps[:], g2row[:], ones1x16[:], start=False,
                         stop=True)
        dk2 = big.tile([16, 16], dt.float32)
        nc.vector.tensor_scalar(out=dk2[:], in0=dk2_ps[:], scalar1=0.0,
                                scalar2=None, op0=Alu.max)
        dk = big.tile([16, 16], dt.float32)
        nc.scalar.activation(out=dk[:], in_=dk2[:], func=Act.Sqrt,
                             bias=b_eps[0:16, 0:1])
        r3 = big.tile([16, 16], dt.float32)
        nc.scalar.activation(out=r3[:], in_=dk[:], func=Act.Relu,
                             bias=b_three[0:16, 0:1], scale=-1.0)
        r3s = big.tile([16, 16], dt.float32)
        nc.scalar.activation(out=r3s[:], in_=r3[:], func=Act.Square)
        val = big.tile([16, 16], dt.float32)
        nc.scalar.activation(out=val[:], in_=r3s[:], func=Act.Ln, bias=1.0)

        nc.vector.tensor_scalar(out=val[:], in0=val[:], scalar1=valid[:, 0:1],
                                scalar2=None, op0=Alu.mult)
        vrow_ps = tiny_ps([1, 16])
        nc.tensor.transpose(vrow_ps[:], valid[:], ident16[:])
        vrow = big.tile([1, 16], dt.float32)
        nc.vector.tensor_copy(out=vrow[:], in_=vrow_ps[:])
        vbc_ps = tiny_ps([16, 16])
        nc.tensor.matmul(vbc_ps[:], ones1x16[:], vrow[:], start=True, stop=True)
        nc.vector.tensor_mul(val[:], val[:], vbc_ps[:])
        nc.vector.tensor_mul(val[:], val[:], iu16[:])

        psr = t16.tile([16, 1], dt.float32)
        nc.vector.tensor_reduce(out=psr[:], in_=val[:], axis=AX.X, op=Alu.add)
        ps_ps = tiny_ps([1, 1])
        nc.tensor.matmul(ps_ps[:], psr[:], ones16[:], start=True, stop=True)
        ps_s = t16.tile([1, 1], dt.float32)
        nc.vector.tensor_copy(out=ps_s[:], in_=ps_ps[:])

        nvm1 = t16.tile([1, 1], dt.float32)
        nc.vector.tensor_scalar(out=nvm1[:], in0=nv_s[:], scalar1=-1.0,
                                scalar2=None, op0=Alu.add)
        den = t16.tile([1, 1], dt.float32)
        nc.vector.tensor_mul(den[:], nv_s[:], nvm1[:])
        den_c = t16.tile([1, 1], dt.float32)
        nc.vector.tensor_scalar(out=den_c[:], in0=den[:], scalar1=1.0,
                                scalar2=None, op0=Alu.max)
        rdn = t16.tile([1, 1], dt.float32)
        nc.vector.reciprocal(rdn[:], den_c[:])
        lpush = t16.tile([1, 1], dt.float32)
        nc.vector.tensor_mul(lpush[:], ps_s[:], rdn[:])
        gate = t16.tile([1, 1], dt.float32)
        nc.vector.tensor_scalar(out=gate[:], in0=nv_s[:], scalar1=1.0,
                                scalar2=None, op0=Alu.is_gt)
        nc.vector.tensor_mul(lpush[:], lpush[:], gate[:])


        # ---- phase 2a: D-hat in (j,m) partition layout; per-pixel select via
        # one mask TS + one TT per block; m-sum via PE accumulation; relu(z)
        # lands directly in vpl2I[:, :, 0] ----
        CB, CH = cfg["CB"], cfg["CH"]
        NCB = F // CB
        psum3 = ctx.enter_context(tc.tile_pool(name="ps3", bufs=1, space="PSUM"))
        p3 = psum3.tile([JT * 2, 16 * JT], dt.float32)
        HF = F // 2

        def l_chain_half(h, ltp):
            hlo = h * HF
            t1 = ltp.tile([P, HF], dt.bfloat16, tag="lt1")
            nc.scalar.activation(out=t1[:], in_=vpl2I[:, hlo:hlo + HF, 0],
                                 func=Act.Sqrt, bias=b_eps[:, 0:1])
            t2 = ltp.tile([P, HF], dt.bfloat16, tag="lt0")
            nc.scalar.activation(out=t2[:], in_=t1[:], func=Act.Relu,
                                 bias=b_mhalf[:, 0:1])
            t3 = ltp.tile([P, HF], dt.bfloat16, tag="lt2")
            nc.scalar.activation(out=t3[:], in_=t2[:], func=Act.Square)
            nc.scalar.activation(out=vpl2I[:, hlo:hlo + HF, 0], in_=t3[:],
                                 func=Act.Ln, bias=1.0)

        with tc.tile_pool(name="b_rhs", bufs=2) as rhp, \
             tc.tile_pool(name="b_tt", bufs=1) as ttp, \
             tc.tile_pool(name="b_tr", bufs=2) as trp, \
             tc.tile_pool(name="b_msk", bufs=2) as mkp, \
             tc.tile_pool(name="q_lt", bufs=1) as ltp, \
             tc.tile_pool(name="b_pd", bufs=1, space="PSUM") as pdp, \
             tc.tile_pool(name="b_z", bufs=1, space="PSUM") as zpp:
            for cb in range(NCB):
                lo = cb * CB
                # rhs layout [48 = ch*8+j, 16u, CB]; pixel row = 16j+u
                rhs2 = rhp.tile([48, 16, CB], dt.bfloat16, tag="rhs")
                for ch in range(6):
                    src = (simbf5[:, ch, lo:lo + CB] if ch < 5 else
                           s2bf[:, lo:lo + CB])
                    rhs_eng = nc.scalar if cb == 0 else nc.gpsimd
                    rhs_eng.dma_start(
                        out=rhs2[ch * 8:(ch + 1) * 8, :, :], in_=src)
                # tt replicated into (j,m) partitions: trep[j*16+m, u, f]
                # = tt[16j+u, f]
                tf2 = ttp.tile([8, 16, CB], dt.float8e4, tag="tf")
                nc.sync.dma_start(out=tf2[:], in_=tt8[:, lo:lo + CB])
                trep = trp.tile([P, 16, CB], dt.float8e4, tag="trep")
                tfa = tf2[:]
                nc.sync.dma_start(out=trep[:], in_=bass.AP(
                    tensor=tfa.tensor, offset=tfa.offset,
                    ap=[list(tfa.ap[0]), [0, 16], [1, 16 * CB]]))
                NCH = CB // CH
                for p0 in range(0, NCH, 2):
                    npair = min(2, NCH - p0)
                    masked = mkp.tile([P, 16, 2 * CH], dt.bfloat16, tag="msk")
                    for o in range(p0, p0 + npair):
                        olo = o * CH
                        # u-halves with alternating PSUM tags: the D-hat
                        # matmuls of one half overlap the select STT of the
                        # other
                        for hu in range(2):
                            u0 = hu * 8
                            pd_h = pdp.tile([P, 8, CH], dt.float32,
                                            tag=f"pd{hu}")
                            # one matmul per 4-u group: moving = [48, 4u, CH]
                            # (2 free dims), out = 512 f32 = one PSUM bank
                            for qg in range(2):
                                nc.tensor.matmul(
                                    pd_h[:, qg * 4:(qg + 1) * 4, :],
                                    gblock[:],
                                    rhs2[:, u0 + qg * 4:u0 + (qg + 1) * 4,
                                         olo:olo + CH],
                                    start=True, stop=True)
                            # masked = (trep == m_of_partition) * pd
                            plo = (o - p0) * CH
                            nc.vector.scalar_tensor_tensor(
                                out=masked[:, u0:u0 + 8, plo:plo + CH],
                                in0=trep[:, u0:u0 + 8, olo:olo + CH],
                                scalar=mvec[:, 0:1],
                                in1=pd_h[:], op0=Alu.is_equal, op1=Alu.mult)
                    # m-sum for this o-pair: each block alone in a PSUM bank
                    # (512-elem pad); u-outer keeps each w8all stationary
                    # loaded across the pair.
                    zps = zpp.tile([P, 2, 512], dt.float32, tag="zps")
                    for u in range(16):
                        for o in range(p0, p0 + npair):
                            po = (o - p0) * CH
                            nc.tensor.matmul(
                                zps[:, o - p0, 0:CH], w8all[:, u, :],
                                masked[:, u, po:po + CH],
                                start=(u == 0), stop=(u == 15))
                    zv = zps[:]
                    zread = bass.AP(tensor=zv.tensor, offset=zv.offset,
                                    ap=[list(zv.ap[0]), [512, npair], [1, CH]])
                    nc.scalar.activation(
                        out=vpl2I[:, lo + p0 * CH:lo + (p0 + npair) * CH, 0],
                        in_=zread, func=Act.Relu)
                done = (cb + 1) * CB
                if done - CB < HF <= done:
                    l_chain_half(0, ltp)
                if cb == NCB - 1:
                    l_chain_half(1, ltp)

        # ---- phase 2b: ohtt + seg-sums per t-chunk ----
        FQt = cfg["FQT"]
        NQt = F // FQt
        NGT = FQt // JT
        with tc.tile_pool(name="q_oh", bufs=2) as ohp:
            for q in range(NQt):
                qlo = q * FQt
                ohtt = ohp.tile([P, M, FQt], dt.bfloat16, tag="ohtt")
                for m in range(M):
                    nc.vector.tensor_scalar(
                        out=ohtt[:, m, :], in0=ttbf[:, qlo:qlo + FQt],
                        scalar1=float(m), scalar2=None, op0=Alu.is_equal)
                # seg-sum over tt for this chunk (PE)
                for t in range(NGT):
                    g = qlo + t * JT
                    lhsT = vpl2I[:, g:g + JT, :]
                    rhs = ohtt[:, :, t * JT:(t + 1) * JT]
                    nc.tensor.matmul(p3[:], lhsT, rhs,
                                     start=(g == 0), stop=(g == F - JT))

        # tt-family diagonal extraction (same scheme as phase 1)
        p3m = big.tile([JT * 2, 16 * JT], dt.float32)
        nc.vector.tensor_mul(p3m[:], p3[:], dselt[:])
        stC_ps = tiny_ps([2, 16 * JT])
        nc.tensor.matmul(stC_ps[:], c2[:], p3m[:], start=True, stop=True)
        stC = big.tile([2, 16], dt.float32)
        nc.vector.tensor_reduce(
            out=stC[:],
            in_=stC_ps.rearrange("p (m j) -> p m j", j=JT),
            axis=AX.X, op=Alu.add)
        stT_ps = tiny_ps([16, 2])
        nc.tensor.transpose(stT_ps[:], stC[:], ident16[0:2, 0:2])
        st = big.tile([16, 2], dt.float32)
        nc.vector.tensor_copy(out=st[:], in_=stT_ps[:])

        # ---- pull loss ----
        cntt_c = t16.tile([16, 1], dt.float32)
        nc.vector.tensor_scalar(out=cntt_c[:], in0=st[:, 1:2], scalar1=1.0,
                                scalar2=None, op0=Alu.max)
        rct = t16.tile([16, 1], dt.float32)
        nc.vector.reciprocal(rct[:], cntt_c[:])
        pim = t16.tile([16, 1], dt.float32)
        nc.vector.tensor_mul(pim[:], st[:, 0:1], rct[:])
        nc.vector.tensor_mul(pim[:], pim[:], valid[:])

        num_ps = tiny_ps([1, 1])
        nc.tensor.matmul(num_ps[:], pim[:], ones16[:], start=True, stop=True)
        num_s = t16.tile([1, 1], dt.float32)
        nc.vector.tensor_copy(out=num_s[:], in_=num_ps[:])
        lpull = t16.tile([1, 1], dt.float32)
        nc.vector.tensor_mul(lpull[:], num_s[:], rnv[:])

        outt = t16.tile([1, 2], dt.float32)
        nc.vector.tensor_copy(out=outt[:, 0:1], in_=lpull[:])
        nc.vector.tensor_copy(out=outt[:, 1:2], in_=lpush[:])
        nc.sync.dma_start(out=out_d.ap(), in_=outt[:])

    nc.compile()
    return nc


def _consts(cfg):
    import ml_dtypes
    bf16 = ml_dtypes.bfloat16
    JK, JT = cfg["JK"], cfg["JT"]
    ident16 = np.eye(16, dtype=np.float32)
    iu16 = np.triu(np.ones((16, 16), np.float32), 1)
    mge1 = (np.arange(16) >= 1).astype(np.float32).reshape(16, 1)
    e6 = np.zeros((6, 48), bf16)
    for ch in range(6):
        e6[ch, ch * 8:(ch + 1) * 8] = 1
    # col c = j*16 + m (j = slot of 16 pixel rows); row r = ch*8 + j'
    dmask48 = np.zeros((48, 128), bf16)
    for r in range(48):
        for cc in range(128):
            if r % 8 == cc // 16:
                dmask48[r, cc] = 1
    mvec = (np.arange(128) % 16).astype(np.float32).reshape(128, 1)
    w8all = np.zeros((128, 16, 128), bf16)
    for p in range(128):
        j = p // 16
        for u in range(16):
            w8all[p, u, 16 * j + u] = 1
    dselk = np.zeros((JK * 5, 16 * JK), bf16)
    for J in range(JK):
        dselk[J * 5:(J + 1) * 5, J::JK] = 1
    c5 = np.zeros((JK * 5, 5), np.float32)
    for J in range(JK):
        for c in range(5):
            c5[J * 5 + c, c] = 1
    dselt = np.zeros((JT * 2, 16 * JT), bf16)
    for J in range(JT):
        dselt[J * 2:(J + 1) * 2, J::JT] = 1
    c2 = np.zeros((JT * 2, 2), np.float32)
    for J in range(JT):
        for c in range(2):
            c2[J * 2 + c, c] = 1
    return dict(ident16=ident16, iu16=iu16, mge1=mge1, e6=e6,
                dmask48=dmask48, dselk=dselk, c5=c5, dselt=dselt, c2=c2,
                mvec=mvec, w8all=w8all)


def make_in_maps(outputs, gt_texts, gt_kernels, cfg):
    P, F = cfg["P"], cfg["F"]
    B = outputs.shape[0]
    consts = _consts(cfg)
    in_maps = []
    for b in range(B):
        sim = np.ascontiguousarray(outputs[b, 4:8], dtype=np.float32)
        in_maps.append(dict(
            sim=sim.reshape(4, P, F),
            kt=np.ascontiguousarray(gt_kernels[b], dtype=np.int32).reshape(P, F),
            tt=np.ascontiguousarray(gt_texts[b], dtype=np.int32).reshape(P, F),
            **consts,
        ))
    return in_maps


def kernel(outputs, gt_texts, gt_kernels, gt_tops=None, gt_bots=None):
    from concourse import bass_utils
    outputs = np.asarray(outputs)
    gt_texts = np.asarray(gt_texts)
    gt_kernels = np.asarray(gt_kernels)
    B = outputs.shape[0]
    cfg = _cfg(outputs.shape[2], outputs.shape[3])
    key = (cfg["H"], cfg["W"])
    if key not in _CACHE:
        _CACHE[key] = build(cfg, for_sim=False)
    nc = _CACHE[key]
    in_maps = make_in_maps(outputs, gt_texts, gt_kernels, cfg)
    res = bass_utils.run_bass_kernel_spmd(nc, in_maps, core_ids=list(range(B)))
    lpull = np.array([res.results[b]["out"][0, 0] for b in range(B)], np.float32)
    lpush = np.array([res.results[b]["out"][0, 1] for b in range(B)], np.float32)
    return lpull, lpush

